# revision 57
# baseline (speedup 1.0000x reference)
"""Trainium2 Bass kernel for nn_CustomLayerMKM: y = x @ (sum_k kron(Bk, Ak)).T + bias.

Exploits the Kronecker structure: per factor, y_b = Bk @ X_b @ Ak^T via two
matmul stages with a DMA-xbar corner-turn between them (~9x fewer FLOPs than
dense).

Sharding: data-parallel over B across 8 cores (512 rows each, processed as 4
b-quarters of 128); Kronecker factors replicated. No collectives.

v2 layout (vs v1): stage-2 uses pattern-stationary matmuls whose output
partition is s = o//32 and PSUM tile r = o%32, letting all 3 factors
accumulate into ONE psum bank (single start flag; has_written semantics give
overwrite-then-accumulate). This cuts stage-2 evictions 3x, removes per-u
weight reloads, and the output is written bf16 (y^T staging layout, host
reassembles + bias).

Index map per factor k with wa:(p,q), wb:(f2,f1), j=i_full//q_k, l=i_full%q_k:
  stage-1 psum col (per i-block t) c = r*4 + e, e encodes (ko//32, j%G)
  U free = r*384 + k*128 + t*4 + e; per-128-block DMA transpose -> V
  V partition p' = t*4+e = {2j+kob | 4j+kob | j} for k={0,1,2}, ko=kob*32+r
  stage-2: yps[s, b] += patB2_k.T @ V[:, r*3+k, :], s = o//32, o = 32s + r

Hardware scheduling constraints baked into the structure (all verified on
trn2): concurrent xbar transposes on both HWDGE rings corrupt SBUF, and any
SBUF-writing DMA concurrent with a transpose is unsafe -> all x loads and
transposes share the sync ring (FIFO-serialized). Every cross-queue DMA is
scheduler-serialized against in-flight transposes with multi-us latency, so
the y stores carry an artificial data dependency (pad columns reading the
last V block) pinning them after the final transpose, on the scalar ring.
The PE/DVE/ACT queues are in-order: quarters are software-pipelined with
2-deep stage-1 lookahead and separate stage-1/stage-2 PSUM pools so no
engine head-of-line blocks on a not-yet-transposed V.
"""

from contextlib import ExitStack

import numpy as np

P = 128
B_FULL, I_DIM, O_DIM = 4096, 4096, 4096
N_CORES = 8
B_SHARD = B_FULL // N_CORES          # 512 rows per core
NQ = 4                               # b-shard processed in 4 quarters of 128
N_FAC = 3
TB = I_DIM // P                      # 32 i-blocks
MM_DTYPE = "bfloat16"


def build_nc(tp_mode="sync", s2_mode="fused"):
    """tp_mode: 'split' (half-tiles, alternate sync/scalar HWDGE),
    'alt' (full tiles, alternate engines), 'sync' (full tiles, sync only).
    s2_mode: 'fused' (one psum group for all 3 factors, single start) or
    'groups' (per-rl accumulation groups, baseline-proven semantics)."""
    import concourse.bass as bass
    import concourse.mybir as mybir
    import concourse.tile as tile
    from concourse import bacc

    MM_DT = getattr(mybir.dt, MM_DTYPE)
    F32 = mybir.dt.float32
    ts = bass.ts

    nc = bacc.Bacc("TRN2", target_bir_lowering=False, debug=False,
                   num_devices=N_CORES)

    # xT laid out [q, pp, t, b]: the per-quarter load is a linear 1MB DMA
    xT_ext = nc.dram_tensor("xT", [NQ, P, TB * P], MM_DT,
                            kind="ExternalInput").ap()
    pat_ext = {}
    for k in range(N_FAC):
        for nm in ("patA", "patB"):
            pat_ext[f"{nm}{k}"] = nc.dram_tensor(
                f"{nm}{k}", [P, P], MM_DT, kind="ExternalInput").ap()
    # y staging: [q, s, r*128 + b (+4 pad)] bf16; host: y[q*128+b, 32s+r].
    # The 4 pad columns carry an artificial dependency on the last corner-
    # turn piece so the y stores never preempt (and stall) a transpose.
    y_ext = nc.dram_tensor("y", [NQ, P, TB * P + 4], MM_DT,
                           kind="ExternalOutput").ap()

    with tile.TileContext(nc) as tc, ExitStack() as ctx:
        const = ctx.enter_context(tc.tile_pool(name="const", bufs=1))
        ps1 = ctx.enter_context(tc.tile_pool(name="ps1", bufs=2, space="PSUM"))
        ps2 = ctx.enter_context(tc.tile_pool(name="ps2", bufs=2, space="PSUM"))
        xtp = ctx.enter_context(tc.tile_pool(name="xtp", bufs=4))
        upool = ctx.enter_context(tc.tile_pool(name="upool", bufs=3))
        vpool = ctx.enter_context(tc.tile_pool(name="vpool", bufs=2))
        ypool = ctx.enter_context(tc.tile_pool(name="ypool", bufs=4))

        def load_patterns():
            patA, patB = [], []
            for k in range(N_FAC):
                pa = const.tile([P, P], MM_DT, tag=f"patA{k}")
                nc.sync.dma_start(pa[:], pat_ext[f"patA{k}"][:])
                pb = const.tile([P, P], MM_DT, tag=f"patB{k}")
                nc.sync.dma_start(pb[:], pat_ext[f"patB{k}"][:])
                patA.append(pa)
                patB.append(pb)
            return patA, patB

        n_ev = [0]

        def evict(dst, src):
            if n_ev[0] % 2 == 0:
                nc.vector.tensor_copy(dst, src)
            else:
                nc.scalar.copy(dst, src)
            n_ev[0] += 1

        n_tp = [0]

        def dma_transpose(dst, src):
            eng = nc.sync if (tp_mode == "sync" or n_tp[0] % 2 == 0) else nc.scalar
            eng.dma_start_transpose(dst, src)
            n_tp[0] += 1

        def load_x(q, pieces=1):
            # sync ring, same as transposes: SBUF-writing DMAs must never
            # run concurrently with the xbar-transpose S2M path (HW hazard);
            # same-ring FIFO serializes them. All 4 quarters are prefetched
            # up front so transposes have the ring to themselves afterward.
            xT_sb = xtp.tile([P, TB, P], MM_DT, tag="xT", name=f"xT{q}")
            w = TB // pieces
            for pc in range(pieces):
                nc.sync.dma_start(xT_sb[:, pc * w:(pc + 1) * w],
                                  xT_ext[q][:, pc * w * P:(pc + 1) * w * P])
            return xT_sb

        def stage1(q, xT_sb, tp_pieces=2):
            # one U slab for all 3 factors -> one transpose pair per quarter
            U = upool.tile([P, N_FAC * TB * P], MM_DT, tag="U", name=f"U{q}")
            for T in range(TB // 4):
                # one psum tile spanning 3 banks (one per factor): a single
                # merged eviction is ~40% cheaper per column than 3 separate
                s1 = ps1.tile([P, N_FAC * 512], F32, tag="ps",
                              name=f"s1_{q}_{T}")
                for tl in range(4):
                    lhsT = xT_sb[:, 4 * T + tl, :]
                    for k in range(N_FAC):
                        nc.tensor.matmul(s1[:, k * 512 + tl * P:
                                            k * 512 + (tl + 1) * P], lhsT,
                                         patA[k][:], start=True, stop=True)
                # evict: U[b, r*384 + k*128 + t*4 + e] = s1[b, k*512+tl*128+r*4+e]
                # iterate (r, k, tl, e): dst runs are 16 contiguous elems (32B)
                src = s1.rearrange("p (k tl r e) -> p r k tl e",
                                   k=N_FAC, tl=4, r=32, e=4)
                dst = U.rearrange("p (r k t e) -> p r k t e",
                                  k=N_FAC, r=32, t=TB,
                                  e=4)[:, :, :, 4 * T:4 * T + 4]
                evict(dst, src)
            # corner-turn; r-pieces so stage-2 R-quads start before the rest
            V = vpool.tile([P, TB * N_FAC, P], MM_DT, tag="V", name=f"V{q}")
            w = TB * N_FAC // tp_pieces
            for pc in range(tp_pieces):
                dma_transpose(V[:, pc * w:(pc + 1) * w],
                              U[:, pc * w * P:(pc + 1) * w * P])
            return V

        def stage2(q, V):
            y_q = ypool.tile([P, TB * P + 4], MM_DT, tag="yq", name=f"yq{q}")
            for R in range(8):
                yps = ps2.tile([P, 512], F32, tag="ps2", name=f"yps{q}_{R}")
                if s2_mode == "fused":
                    for k in range(N_FAC):
                        for rl in range(4):
                            r = R * 4 + rl
                            nc.tensor.matmul(
                                yps[:, ts(rl, P)],
                                patB[k][:],
                                V[:, r * N_FAC + k, :],
                                start=(k == 0 and rl == 0),
                                stop=(k == N_FAC - 1 and rl == 3),
                                skip_group_check=True)
                else:
                    for rl in range(4):
                        r = R * 4 + rl
                        for k in range(N_FAC):
                            nc.tensor.matmul(
                                yps[:, ts(rl, P)],
                                patB[k][:],
                                V[:, r * N_FAC + k, :],
                                start=(k == 0),
                                stop=(k == N_FAC - 1))
                evict(y_q[:, ts(R, 512)], yps[:])
            return y_q

        # Software pipeline, 2-deep stage-1 lookahead; separate psum pools
        # (ps1/ps2) keep stage-2 psum tiles off stage-1's recycle ring so the
        # in-order engine queues never head-of-line block on late V tiles.
        xT = [None] * NQ
        xT[0] = load_x(0, pieces=4)
        patA, patB = load_patterns()
        for q in range(1, NQ):
            xT[q] = load_x(q)
        V = [None] * NQ
        V[0] = stage1(0, xT[0])
        V[1] = stage1(1, xT[1])
        V[2] = stage1(2, xT[2])
        yq = [None] * NQ
        yq[0] = stage2(0, V[0])
        V[3] = stage1(3, xT[3], tp_pieces=4)
        yq[1] = stage2(1, V[1])
        yq[2] = stage2(2, V[2])
        yq[3] = stage2(3, V[3])
        # pin the y stores behind the final corner-turn piece: the pad-column
        # copy reads V(3)'s last block, so the store (which covers the pad)
        # cannot be hoisted by the scheduler into a transpose window
        for q in range(NQ):
            nc.vector.tensor_copy(yq[q][:, TB * P:], V[3][:, TB * N_FAC - 1, :4])
            # alternate queues so the 4 transfers overlap (plain DMAs on
            # different queues are not serialized against each other)
            eng = nc.scalar if q % 2 == 0 else nc.gpsimd
            eng.dma_start(y_ext[q], yq[q][:])

    nc.compile()
    return nc


_NC_CACHE = {}


def prep_inputs(inputs):
    """Host preprocessing: per-core bf16 quarter-major xT + pattern matrices."""
    import ml_dtypes

    bf16 = ml_dtypes.bfloat16
    x = np.asarray(inputs["input_BI"], dtype=np.float32)
    As = [np.asarray(inputs[n], dtype=np.float32) for n in ("w0a", "w1a", "w2a")]
    Bs = [np.asarray(inputs[n], dtype=np.float32) for n in ("w0b", "w1b", "w2b")]

    common = {}
    # patA_k[pp, r*4+e]; see module docstring for the index map
    pa0 = np.zeros((2, 64, 32, 2, 2), np.float32)      # [g, l, r, g', kob]
    w0 = As[0].reshape(2, 32, 64).transpose(2, 1, 0)   # [l, r, kob]
    for g in range(2):
        pa0[g, :, :, g, :] = w0
    common["patA0"] = pa0.reshape(P, P)
    common["patA1"] = As[1].reshape(4, 32, P).transpose(2, 1, 0).reshape(P, P)
    pa2 = np.zeros((4, 32, 32, 4), np.float32)         # [g, l, r, g']
    for g in range(4):
        pa2[g, :, :, g] = As[2].T                       # [l, r] (ko = r)
    common["patA2"] = pa2.reshape(P, P)

    # patB2_k[p', s]
    pb0 = np.zeros((64, 2, 64, 2), np.float32)         # [j, kob, i, kob']
    for kob in range(2):
        pb0[:, kob, :, kob] = Bs[0].T
    common["patB0"] = pb0.reshape(P, P)
    pb1 = np.zeros((32, 4, 32, 4), np.float32)
    for kob in range(4):
        pb1[:, kob, :, kob] = Bs[1].T
    common["patB1"] = pb1.reshape(P, P)
    common["patB2"] = np.ascontiguousarray(Bs[2].T)

    for k in list(common):
        common[k] = np.ascontiguousarray(common[k].astype(bf16))

    in_maps = []
    for c in range(N_CORES):
        im = dict(common)
        xs = x[c * B_SHARD:(c + 1) * B_SHARD].T.astype(bf16)   # (4096, 512)
        # [q, pp, t, b]: per-quarter DMA is a linear [128, 4096] copy
        im["xT"] = np.ascontiguousarray(
            xs.reshape(TB, P, NQ, P).transpose(2, 1, 0, 3)
            .reshape(NQ, P, TB * P))
        in_maps.append(im)
    return in_maps


def finish_output(res_list, bias):
    """Reassemble [q,s,r,b] bf16 staging -> [B, O] f32 + bias."""
    outs = []
    for r in res_list:
        ystage = np.asarray(r["y"]).reshape(NQ, P, TB * P + 4)
        ystage = ystage[:, :, :TB * P].reshape(NQ, P, TB, P)
        y_core = ystage.transpose(0, 3, 1, 2).reshape(B_SHARD, O_DIM)
        outs.append(y_core.astype(np.float32))
    y = np.concatenate(outs, axis=0)
    return y + bias[None, :]


def kernel(**inputs):
    """Full-input entry point: shards over B, runs 8-core SPMD, gathers."""
    from concourse.bass_utils import run_bass_kernel_spmd

    in_maps = prep_inputs(inputs)
    if "nc" not in _NC_CACHE:
        _NC_CACHE["nc"] = build_nc()
    res = run_bass_kernel_spmd(_NC_CACHE["nc"], in_maps,
                               core_ids=list(range(N_CORES)))
    bias = np.asarray(inputs["bias_O"], dtype=np.float32)
    return finish_output(res.results, bias)


# revision 58
# speedup vs baseline: 1.0497x; 1.0497x over previous
"""Trainium2 Bass kernel for nn_CustomLayerMKM: y = x @ (sum_k kron(Bk, Ak)).T + bias.

Exploits the Kronecker structure: per factor, y_b = Bk @ X_b @ Ak^T via two
matmul stages with a DMA-xbar corner-turn between them (~9x fewer FLOPs than
dense).

Sharding: data-parallel over B across 8 cores (512 rows each, processed as 4
b-quarters of 128); Kronecker factors replicated. No collectives.

v2 layout (vs v1): stage-2 uses pattern-stationary matmuls whose output
partition is s = o//32 and PSUM tile r = o%32, letting all 3 factors
accumulate into ONE psum bank (single start flag; has_written semantics give
overwrite-then-accumulate). This cuts stage-2 evictions 3x, removes per-u
weight reloads, and the output is written bf16 (y^T staging layout, host
reassembles + bias).

Index map per factor k with wa:(p,q), wb:(f2,f1), j=i_full//q_k, l=i_full%q_k:
  stage-1 psum col (per i-block t) c = r*4 + e, e encodes (ko//32, j%G)
  U free = r*384 + k*128 + t*4 + e; per-128-block DMA transpose -> V
  V partition p' = t*4+e = {2j+kob | 4j+kob | j} for k={0,1,2}, ko=kob*32+r
  stage-2: yps[s, b] += patB2_k.T @ V[:, r*3+k, :], s = o//32, o = 32s + r

Hardware scheduling constraints baked into the structure (all verified on
trn2): concurrent xbar transposes on both HWDGE rings corrupt SBUF, and any
SBUF-writing DMA concurrent with a transpose is unsafe -> all x loads and
transposes share the sync ring (FIFO-serialized). Every cross-queue DMA is
scheduler-serialized against in-flight transposes with multi-us latency, so
the y stores carry an artificial data dependency (pad columns reading the
last V block) pinning them after the final transpose, on the scalar ring.
The PE/DVE/ACT queues are in-order: quarters are software-pipelined with
2-deep stage-1 lookahead and separate stage-1/stage-2 PSUM pools so no
engine head-of-line blocks on a not-yet-transposed V.
"""

from contextlib import ExitStack

import numpy as np

P = 128
B_FULL, I_DIM, O_DIM = 4096, 4096, 4096
N_CORES = 8
B_SHARD = B_FULL // N_CORES          # 512 rows per core
NQ = 4                               # b-shard processed in 4 quarters of 128
N_FAC = 3
TB = I_DIM // P                      # 32 i-blocks
MM_DTYPE = "bfloat16"


def build_nc(tp_mode="sync", s2_mode="fused"):
    """tp_mode: 'split' (half-tiles, alternate sync/scalar HWDGE),
    'alt' (full tiles, alternate engines), 'sync' (full tiles, sync only).
    s2_mode: 'fused' (one psum group for all 3 factors, single start) or
    'groups' (per-rl accumulation groups, baseline-proven semantics)."""
    import concourse.bass as bass
    import concourse.mybir as mybir
    import concourse.tile as tile
    from concourse import bacc

    MM_DT = getattr(mybir.dt, MM_DTYPE)
    F32 = mybir.dt.float32
    ts = bass.ts

    nc = bacc.Bacc("TRN2", target_bir_lowering=False, debug=False,
                   num_devices=N_CORES)

    # xT laid out [q, pp, t, b]: the per-quarter load is a linear 1MB DMA
    xT_ext = nc.dram_tensor("xT", [NQ, P, TB * P], MM_DT,
                            kind="ExternalInput").ap()
    pat_ext = {}
    for k in range(N_FAC):
        for nm in ("patA", "patB"):
            pat_ext[f"{nm}{k}"] = nc.dram_tensor(
                f"{nm}{k}", [P, P], MM_DT, kind="ExternalInput").ap()
    # y staging: [q, s, r*128 + b (+4 pad)] bf16; host: y[q*128+b, 32s+r].
    # The 4 pad columns carry an artificial dependency on the last corner-
    # turn piece so the y stores never preempt (and stall) a transpose.
    y_ext = nc.dram_tensor("y", [NQ, P, TB * P + 4], MM_DT,
                           kind="ExternalOutput").ap()

    with tile.TileContext(nc) as tc, ExitStack() as ctx:
        const = ctx.enter_context(tc.tile_pool(name="const", bufs=1))
        ps1 = ctx.enter_context(tc.tile_pool(name="ps1", bufs=6, space="PSUM"))
        ps2 = ctx.enter_context(tc.tile_pool(name="ps2", bufs=2, space="PSUM"))
        xtp = ctx.enter_context(tc.tile_pool(name="xtp", bufs=4))
        upool = ctx.enter_context(tc.tile_pool(name="upool", bufs=3))
        vpool = ctx.enter_context(tc.tile_pool(name="vpool", bufs=2))
        ypool = ctx.enter_context(tc.tile_pool(name="ypool", bufs=4))

        def load_patterns():
            patA, patB = [], []
            for k in range(N_FAC):
                pa = const.tile([P, P], MM_DT, tag=f"patA{k}")
                nc.sync.dma_start(pa[:], pat_ext[f"patA{k}"][:])
                pb = const.tile([P, P], MM_DT, tag=f"patB{k}")
                nc.sync.dma_start(pb[:], pat_ext[f"patB{k}"][:])
                patA.append(pa)
                patB.append(pb)
            return patA, patB

        n_ev = [0]

        def evict(dst, src):
            if n_ev[0] % 2 == 0:
                nc.vector.tensor_copy(dst, src)
            else:
                nc.scalar.copy(dst, src)
            n_ev[0] += 1

        n_tp = [0]

        def dma_transpose(dst, src):
            eng = nc.sync if (tp_mode == "sync" or n_tp[0] % 2 == 0) else nc.scalar
            eng.dma_start_transpose(dst, src)
            n_tp[0] += 1

        def load_x(q, pieces=1):
            # sync ring, same as transposes: SBUF-writing DMAs must never
            # run concurrently with the xbar-transpose S2M path (HW hazard);
            # same-ring FIFO serializes them. All 4 quarters are prefetched
            # up front so transposes have the ring to themselves afterward.
            xT_sb = xtp.tile([P, TB, P], MM_DT, tag="xT", name=f"xT{q}")
            w = TB // pieces
            for pc in range(pieces):
                nc.sync.dma_start(xT_sb[:, pc * w:(pc + 1) * w],
                                  xT_ext[q][:, pc * w * P:(pc + 1) * w * P])
            return xT_sb

        def stage1(q, xT_sb, tp_pieces=2):
            # one U slab for all 3 factors -> one transpose pair per quarter
            U = upool.tile([P, N_FAC * TB * P], MM_DT, tag="U", name=f"U{q}")
            for T in range(TB // 4):
                s1 = [ps1.tile([P, 512], F32, tag="ps",
                               name=f"s1_{q}_{T}_{kk}")
                      for kk in range(N_FAC)]
                for tl in range(4):
                    lhsT = xT_sb[:, 4 * T + tl, :]
                    for k in range(N_FAC):
                        nc.tensor.matmul(s1[k][:, ts(tl, P)], lhsT,
                                         patA[k][:], start=True, stop=True)
                # evict: U[b, r*384 + k*128 + t*4 + e] = s1_k[b, tl*128+r*4+e]
                # iterate (r, tl, e): dst runs are 16 contiguous elems (32B)
                for k in range(N_FAC):
                    src = s1[k].rearrange("p (tl r e) -> p r tl e",
                                          tl=4, r=32, e=4)
                    dst = U.rearrange("p (r k t e) -> p r k t e",
                                      k=N_FAC, r=32, t=TB,
                                      e=4)[:, :, k, 4 * T:4 * T + 4]
                    evict(dst, src)
            # corner-turn; r-pieces so stage-2 R-quads start before the rest
            V = vpool.tile([P, TB * N_FAC, P], MM_DT, tag="V", name=f"V{q}")
            w = TB * N_FAC // tp_pieces
            for pc in range(tp_pieces):
                dma_transpose(V[:, pc * w:(pc + 1) * w],
                              U[:, pc * w * P:(pc + 1) * w * P])
            return V

        def stage2(q, V):
            y_q = ypool.tile([P, TB * P + 4], MM_DT, tag="yq", name=f"yq{q}")
            for R in range(8):
                yps = ps2.tile([P, 512], F32, tag="ps2", name=f"yps{q}_{R}")
                if s2_mode == "fused":
                    for k in range(N_FAC):
                        for rl in range(4):
                            r = R * 4 + rl
                            nc.tensor.matmul(
                                yps[:, ts(rl, P)],
                                patB[k][:],
                                V[:, r * N_FAC + k, :],
                                start=(k == 0 and rl == 0),
                                stop=(k == N_FAC - 1 and rl == 3),
                                skip_group_check=True)
                else:
                    for rl in range(4):
                        r = R * 4 + rl
                        for k in range(N_FAC):
                            nc.tensor.matmul(
                                yps[:, ts(rl, P)],
                                patB[k][:],
                                V[:, r * N_FAC + k, :],
                                start=(k == 0),
                                stop=(k == N_FAC - 1))
                evict(y_q[:, ts(R, 512)], yps[:])
            return y_q

        # Software pipeline, 2-deep stage-1 lookahead; separate psum pools
        # (ps1/ps2) keep stage-2 psum tiles off stage-1's recycle ring so the
        # in-order engine queues never head-of-line block on late V tiles.
        xT = [None] * NQ
        xT[0] = load_x(0, pieces=4)
        patA, patB = load_patterns()
        for q in range(1, NQ):
            xT[q] = load_x(q)
        V = [None] * NQ
        V[0] = stage1(0, xT[0])
        V[1] = stage1(1, xT[1])
        V[2] = stage1(2, xT[2])
        yq = [None] * NQ
        yq[0] = stage2(0, V[0])
        V[3] = stage1(3, xT[3], tp_pieces=4)
        yq[1] = stage2(1, V[1])
        yq[2] = stage2(2, V[2])
        yq[3] = stage2(3, V[3])
        # pin the y stores behind the final corner-turn piece: the pad-column
        # copy reads V(3)'s last block, so the store (which covers the pad)
        # cannot be hoisted by the scheduler into a transpose window
        for q in range(NQ):
            nc.vector.tensor_copy(yq[q][:, TB * P:], V[3][:, TB * N_FAC - 1, :4])
            nc.scalar.dma_start(y_ext[q], yq[q][:])

    nc.compile()
    return nc


_NC_CACHE = {}


def prep_inputs(inputs):
    """Host preprocessing: per-core bf16 quarter-major xT + pattern matrices."""
    import ml_dtypes

    bf16 = ml_dtypes.bfloat16
    x = np.asarray(inputs["input_BI"], dtype=np.float32)
    As = [np.asarray(inputs[n], dtype=np.float32) for n in ("w0a", "w1a", "w2a")]
    Bs = [np.asarray(inputs[n], dtype=np.float32) for n in ("w0b", "w1b", "w2b")]

    common = {}
    # patA_k[pp, r*4+e]; see module docstring for the index map
    pa0 = np.zeros((2, 64, 32, 2, 2), np.float32)      # [g, l, r, g', kob]
    w0 = As[0].reshape(2, 32, 64).transpose(2, 1, 0)   # [l, r, kob]
    for g in range(2):
        pa0[g, :, :, g, :] = w0
    common["patA0"] = pa0.reshape(P, P)
    common["patA1"] = As[1].reshape(4, 32, P).transpose(2, 1, 0).reshape(P, P)
    pa2 = np.zeros((4, 32, 32, 4), np.float32)         # [g, l, r, g']
    for g in range(4):
        pa2[g, :, :, g] = As[2].T                       # [l, r] (ko = r)
    common["patA2"] = pa2.reshape(P, P)

    # patB2_k[p', s]
    pb0 = np.zeros((64, 2, 64, 2), np.float32)         # [j, kob, i, kob']
    for kob in range(2):
        pb0[:, kob, :, kob] = Bs[0].T
    common["patB0"] = pb0.reshape(P, P)
    pb1 = np.zeros((32, 4, 32, 4), np.float32)
    for kob in range(4):
        pb1[:, kob, :, kob] = Bs[1].T
    common["patB1"] = pb1.reshape(P, P)
    common["patB2"] = np.ascontiguousarray(Bs[2].T)

    for k in list(common):
        common[k] = np.ascontiguousarray(common[k].astype(bf16))

    in_maps = []
    for c in range(N_CORES):
        im = dict(common)
        xs = x[c * B_SHARD:(c + 1) * B_SHARD].T.astype(bf16)   # (4096, 512)
        # [q, pp, t, b]: per-quarter DMA is a linear [128, 4096] copy
        im["xT"] = np.ascontiguousarray(
            xs.reshape(TB, P, NQ, P).transpose(2, 1, 0, 3)
            .reshape(NQ, P, TB * P))
        in_maps.append(im)
    return in_maps


def finish_output(res_list, bias):
    """Reassemble [q,s,r,b] bf16 staging -> [B, O] f32 + bias."""
    outs = []
    for r in res_list:
        ystage = np.asarray(r["y"]).reshape(NQ, P, TB * P + 4)
        ystage = ystage[:, :, :TB * P].reshape(NQ, P, TB, P)
        y_core = ystage.transpose(0, 3, 1, 2).reshape(B_SHARD, O_DIM)
        outs.append(y_core.astype(np.float32))
    y = np.concatenate(outs, axis=0)
    return y + bias[None, :]


def kernel(**inputs):
    """Full-input entry point: shards over B, runs 8-core SPMD, gathers."""
    from concourse.bass_utils import run_bass_kernel_spmd

    in_maps = prep_inputs(inputs)
    if "nc" not in _NC_CACHE:
        _NC_CACHE["nc"] = build_nc()
    res = run_bass_kernel_spmd(_NC_CACHE["nc"], in_maps,
                               core_ids=list(range(N_CORES)))
    bias = np.asarray(inputs["bias_O"], dtype=np.float32)
    return finish_output(res.results, bias)
